# revision 4
# baseline (speedup 1.0000x reference)
"""DSVT-to-dense-BEV scatter-max kernel for Trainium2 (8 NeuronCores).

Reference op: scatter-max of voxel features [N,C] into a dense BEV grid
(B, C, NY, NX) keyed by (batch_idx, y_idx, x_idx); empty cells = 0.

Sharding: core k <- (batch b = k//2, y-half h = k%2); each core owns a
[C=128, 200*400=80000] output slab.

Host does index-only prep + feature sharding/sorting/duplication (per the
sharding hint); all feature movement and the max-reduction run on device.

Device algorithm (one-hot matmul scatter; no DRAM scratch, no indirect DMA):
  The slab is processed as 157 blocks of 512 cells, each owning one PSUM
  bank tile [C=128, 512].  For each block the PE computes
      psum = T0_tile.T @ onehot0  +  M_tile.T @ onehot1
  where T0_tile [128 slots, C] holds first-voxel rows of single-voxel
  cells of the block (host-packed, slot-transposed in DRAM so the load is
  one big contiguous DMA), M_tile holds the device max-reduced rows of
  multi-voxel cells of the block's 6-block group, and the one-hot
  matrices are built on DVE via is_equal(iota16, idx) from host-provided
  per-(slot, op) target-column tables.  Each output column receives
  exactly one contributing row (singles XOR multi), so the PSUM sum is an
  exact placement; empty cells stay 0 from the start=True accumulation.
  Multi-cell reduction: M = max(D1, D2) over the packed multi stream
  (pairs + demoted singles), then depth-3.. levels max-folded over the
  first 32 partitions only (hi cells are sorted to the top of each group
  tile).  ACT evacuates PSUM -> SBUF staging with a f32->bf16 cast; big
  HWDGE DMAs store staging -> OUT (bf16); host widens to f32 (exact for
  the bf16-rounded values; rel-err ~2^-9, tolerance 2e-2).

The device body sits in For_i(0, nit) for slope timing (nit=1 normally).
"""

import os
import numpy as np
import ml_dtypes

_KVAR = set(os.environ.get("KVAR", "").split(",")) - {""}

# ---------------- problem constants (hardcoded; kernel.py is standalone) ---
N_VOXELS = 150000
C = 128
NX = 400
NY = 400
B = 4
N_CORES = 8

P = 128
SLAB = 200 * NX              # 80000 cells per core
BS = 512                     # cells per block (one PSUM bank of fp32)
NBLK = (SLAB + BS - 1) // BS  # 157 (last block has 128 cells)
G = 6                        # blocks per multi-cell group tile
NGRP = (NBLK + G - 1) // G   # 27
HI_CAP = 32                  # hi (count>=3) rows live in partitions [0,32)
HID = 3                      # extra reduction levels (voxels 3..5); count<=5
SCHUNK = 8                   # blocks per staging/store chunk
NIT_MAX = 4096

BF16 = ml_dtypes.bfloat16

_cache = {}


def _blk_cells(b):
    return min(BS, SLAB - b * BS)


def _build_nc():
    from concourse import bass, bacc, mybir
    import concourse.tile as tile

    f32 = mybir.dt.float32
    f16 = mybir.dt.float16
    bf16 = mybir.dt.bfloat16
    i32 = mybir.dt.int32

    W = NGRP * C            # free width of one level of the multi tables

    nc = bacc.Bacc(None, target_bir_lowering=False, debug=False)
    T0T = nc.dram_tensor("t0t", [P, NBLK * C], bf16, kind="ExternalInput")
    DP = nc.dram_tensor("dp", [P, 2 * W], bf16, kind="ExternalInput")
    HP = nc.dram_tensor("hp", [HI_CAP, HID * W], bf16, kind="ExternalInput")
    IDX = nc.dram_tensor("idx", [P, 2 * NBLK], f32, kind="ExternalInput")
    IOTA = nc.dram_tensor("iota", [P, BS], f16, kind="ExternalInput")
    NIT = nc.dram_tensor("nit", [1, 2], i32, kind="ExternalInput")
    OUT = nc.dram_tensor("out", [C, SLAB], bf16, kind="ExternalOutput")

    with tile.TileContext(nc) as tc:
        with (
            tc.tile_pool(name="const", bufs=1) as cpool,
            tc.tile_pool(name="t0", bufs=2) as tpool,
            tc.tile_pool(name="dh", bufs=2) as dpool,
            tc.tile_pool(name="rmax", bufs=2) as rpool,
            tc.tile_pool(name="oh", bufs=6) as opool,
            tc.tile_pool(name="stg", bufs=2) as spool,
            tc.tile_pool(name="ps", bufs=8, space="PSUM") as ppool,
        ):
            iota = cpool.tile([P, BS], f16)
            nc.sync.dma_start(iota[:], IOTA[:])
            idx = cpool.tile([P, 2 * NBLK], f32)
            nc.sync.dma_start(idx[:], IDX[:])
            nit_sb = cpool.tile([1, 2], i32)
            nc.sync.dma_start(nit_sb[:], NIT[:])

            nit = nc.values_load(nit_sb[0:1, 0:1], min_val=0, max_val=NIT_MAX,
                                 skip_runtime_bounds_check=True)

            with tc.For_i(0, nit):
                # ---- table loads (big contiguous HWDGE DMAs) -------------
                t0 = tpool.tile([P, NBLK * C], bf16, tag="t0")
                nc.scalar.dma_start(t0[:], T0T[:])
                dp = dpool.tile([P, 2 * W], bf16, tag="dp")
                nc.scalar.dma_start(dp[:], DP[:])
                hp = dpool.tile([HI_CAP, HID * W], bf16, tag="hp")
                nc.scalar.dma_start(hp[:], HP[:])

                # ---- multi-stream reduction ------------------------------
                m = rpool.tile([P, W], bf16, tag="m")
                nc.vector.tensor_tensor(out=m[:], in0=dp[:, 0:W],
                                        in1=dp[:, W:2 * W],
                                        op=mybir.AluOpType.max)
                for j in range(HID):
                    nc.vector.tensor_tensor(
                        out=m[0:HI_CAP, :], in0=m[0:HI_CAP, :],
                        in1=hp[:, j * W:(j + 1) * W],
                        op=mybir.AluOpType.max)

                # ---- per-block one-hot matmul scatter --------------------
                stg = None
                for b in range(NBLK):
                    if b % SCHUNK == 0:
                        cw = sum(_blk_cells(x)
                                 for x in range(b, min(b + SCHUNK, NBLK)))
                        stg = spool.tile([P, SCHUNK * BS], bf16, tag="stg")
                    w = _blk_cells(b)
                    oh0 = opool.tile([P, BS], bf16, tag="oh0")
                    nc.vector.tensor_scalar(
                        out=oh0[:], in0=iota[:], scalar1=idx[:, b:b + 1],
                        scalar2=None, op0=mybir.AluOpType.is_equal)
                    if "oh1" in _KVAR:
                        oh1 = oh0
                    else:
                        oh1 = opool.tile([P, BS], bf16, tag="oh1")
                        nc.vector.tensor_scalar(
                            out=oh1[:], in0=iota[:],
                            scalar1=idx[:, NBLK + b:NBLK + b + 1],
                            scalar2=None, op0=mybir.AluOpType.is_equal)
                    ps = ppool.tile([P, BS], f32, tag="ps")
                    nc.tensor.matmul(ps[:], t0[:, b * C:(b + 1) * C], oh0[:],
                                     start=True, stop=False)
                    g = b // G
                    nc.tensor.matmul(ps[:], m[:, g * C:(g + 1) * C], oh1[:],
                                     start=False, stop=True)
                    off = (b % SCHUNK) * BS
                    if "noact" not in _KVAR:
                        nc.scalar.activation(
                            out=stg[:, off:off + w], in_=ps[:, 0:w],
                            func=mybir.ActivationFunctionType.Copy)
                    if b % SCHUNK == SCHUNK - 1 or b == NBLK - 1:
                        c0 = (b - b % SCHUNK) * BS
                        cw = (b % SCHUNK) * BS + w
                        if "nostore" not in _KVAR:
                            nc.sync.dma_start(OUT[:, c0:c0 + cw],
                                              stg[:, 0:cw])

    nc.compile()
    return nc


def _host_prep(voxel_features, batch_idx, y_idx, x_idx):
    """Index prep + feature sharding/sorting. Returns per-core input maps."""
    vf = np.ascontiguousarray(np.asarray(voxel_features, dtype=np.float32))
    bi = np.asarray(batch_idx, dtype=np.int64)
    yi = np.asarray(y_idx, dtype=np.int64)
    xi = np.asarray(x_idx, dtype=np.int64)

    half = yi >= 200
    core_of = bi * 2 + half
    loccell = (yi - half * 200) * NX + xi

    W = NGRP * C
    iota = np.tile(np.arange(BS, dtype=np.float16), (P, 1))

    in_maps = []
    for k in range(N_CORES):
        vs = np.nonzero(core_of == k)[0]
        cells = loccell[vs]
        order = np.argsort(cells, kind="stable")
        svs = vs[order]
        sc = cells[order]
        uniq, starts, counts = np.unique(sc, return_index=True,
                                         return_counts=True)
        assert counts.max(initial=1) <= 2 + HID, counts.max()

        # --- singles: keep at most 128 per block, demote the rest --------
        is_s = counts == 1
        sidx = uniq[is_s]
        svox = svs[starts[is_s]]
        sblk = sidx // BS
        blk_first = np.searchsorted(sblk, np.arange(NBLK))
        srank = np.arange(len(sidx)) - blk_first[sblk]
        keep = srank < P

        t0t = np.zeros((NBLK, P, C), np.float32)
        t0t[sblk[keep], srank[keep]] = vf[svox[keep]]
        t0idx = np.full((P, NBLK), -3.0, np.float32)
        t0idx[srank[keep], sblk[keep]] = (sidx[keep] % BS).astype(np.float32)

        # --- multi stream: multi cells + demoted singles, grouped --------
        m_cells = np.concatenate([uniq[~is_s], sidx[~keep]])
        m_starts = np.concatenate([starts[~is_s], starts[is_s][~keep]])
        m_counts = np.concatenate([counts[~is_s], np.ones((~keep).sum(),
                                                          np.int64)])
        grp = m_cells // (BS * G)
        ishi = m_counts >= 3
        o2 = np.lexsort((m_cells, ~ishi, grp))
        m_cells, m_starts, m_counts, ishi = (m_cells[o2], m_starts[o2],
                                             m_counts[o2], ishi[o2])
        grp = grp[o2]
        grp_first = np.searchsorted(grp, np.arange(NGRP))
        grank = np.arange(len(m_cells)) - grp_first[grp]
        assert grank.max(initial=0) < P, grank.max()
        assert grank[ishi].max(initial=0) < HI_CAP, grank[ishi].max()

        d1 = np.zeros((NGRP, P, C), np.float32)
        d2 = np.zeros((NGRP, P, C), np.float32)
        d1[grp, grank] = vf[svs[m_starts]]
        d2[grp, grank] = vf[svs[m_starts + np.minimum(1, m_counts - 1)]]
        hps = np.full((HID, NGRP, HI_CAP, C), -3.0e38, np.float32)
        hgrp, hrank = grp[ishi], grank[ishi]
        hst, hcn = m_starts[ishi], m_counts[ishi]
        for j in range(HID):
            hps[j, hgrp, hrank] = vf[svs[hst + np.minimum(2 + j, hcn - 1)]]

        ridx = np.full((P, NBLK), -3.0, np.float32)
        ridx[grank, m_cells // BS] = (m_cells % BS).astype(np.float32)

        in_maps.append({
            "t0t": np.ascontiguousarray(
                t0t.transpose(1, 0, 2).reshape(P, NBLK * C)).astype(BF16),
            "dp": np.ascontiguousarray(np.concatenate(
                [d1.transpose(1, 0, 2).reshape(P, W),
                 d2.transpose(1, 0, 2).reshape(P, W)], axis=1)).astype(BF16),
            "hp": np.ascontiguousarray(np.concatenate(
                [hps[j].transpose(1, 0, 2).reshape(HI_CAP, W)
                 for j in range(HID)], axis=1)).astype(BF16),
            "idx": np.ascontiguousarray(
                np.concatenate([t0idx, ridx], axis=1)),
            "iota": iota,
            "nit": np.array([[1, 0]], np.int32),
        })
    return in_maps


class _Runner:
    """Cached-jit SPMD runner (mirrors bass2jax.run_bass_via_pjrt)."""

    def __init__(self, nc, n_cores=N_CORES):
        import jax
        from jax.sharding import Mesh, PartitionSpec, NamedSharding
        from jax.experimental.shard_map import shard_map
        from concourse import mybir
        from concourse.bass2jax import (_bass_exec_p, install_neuronx_cc_hook,
                                        partition_id_tensor)

        install_neuronx_cc_hook()
        self.jax = jax
        partition_name = (nc.partition_id_tensor.name
                          if nc.partition_id_tensor else None)
        in_names, out_names, out_avals, zero_outs = [], [], [], []
        for alloc in nc.m.functions[0].allocations:
            if not isinstance(alloc, mybir.MemoryLocationSet):
                continue
            name = alloc.memorylocations[0].name
            if alloc.kind == "ExternalInput":
                if name != partition_name:
                    in_names.append(name)
            elif alloc.kind == "ExternalOutput":
                shape = tuple(alloc.tensor_shape)
                dtype = mybir.dt.np(alloc.dtype)
                out_names.append(name)
                out_avals.append(jax.core.ShapedArray(shape, dtype))
                zero_outs.append(np.zeros(shape, dtype))
        self.in_names, self.out_names = in_names, out_names
        self.out_avals, self.zero_outs = out_avals, zero_outs
        self.n_cores = n_cores
        n_params, n_outs = len(in_names), len(out_avals)
        all_in = list(in_names) + list(out_names)
        if partition_name is not None:
            all_in.append(partition_name)

        def _body(*args):
            operands = list(args)
            if partition_name is not None:
                operands.append(partition_id_tensor())
            return tuple(_bass_exec_p.bind(
                *operands, out_avals=tuple(out_avals), in_names=tuple(all_in),
                out_names=tuple(out_names), lowering_input_output_aliases=(),
                sim_require_finite=True, sim_require_nnan=True, nc=nc))

        devices = jax.devices()[:n_cores]
        self.mesh = Mesh(np.asarray(devices), ("core",))
        self.sh = NamedSharding(self.mesh, PartitionSpec("core"))
        self._fn = jax.jit(
            shard_map(_body, mesh=self.mesh,
                      in_specs=(PartitionSpec("core"),) * (n_params + n_outs),
                      out_specs=(PartitionSpec("core"),) * n_outs,
                      check_rep=False),
            donate_argnums=tuple(range(n_params, n_params + n_outs)),
            keep_unused=True)
        self._dev_inputs = None
        self._out_bufs = None

    def set_inputs(self, in_maps):
        self._dev_inputs = [
            self.jax.device_put(
                np.concatenate([np.asarray(m[name]) for m in in_maps], axis=0),
                self.sh)
            for name in self.in_names
        ]
        self._out_bufs = None

    def update_input(self, name, arrays):
        i = self.in_names.index(name)
        self._dev_inputs[i] = self.jax.device_put(
            np.concatenate([np.asarray(a) for a in arrays], axis=0), self.sh)

    def run(self):
        if self._out_bufs is None:
            self._out_bufs = [
                self.jax.device_put(
                    np.zeros((self.n_cores * z.shape[0], *z.shape[1:]),
                             z.dtype), self.sh)
                for z in self.zero_outs
            ]
        outs = self._fn(*self._dev_inputs, *self._out_bufs)
        self._out_bufs = list(outs)
        return outs

    def block(self):
        for o in self._out_bufs:
            o.block_until_ready()

    def fetch(self, name):
        i = self.out_names.index(name)
        arr = np.asarray(self._out_bufs[i])
        return arr.reshape(self.n_cores, *self.out_avals[i].shape)


def _get_runner():
    if "runner" not in _cache:
        nc = _build_nc()
        _cache["nc"] = nc
        _cache["runner"] = _Runner(nc)
    return _cache["runner"]


def kernel(voxel_features, batch_idx, y_idx, x_idx, batch_size):
    bs = int(np.asarray(batch_size))
    assert bs == B
    in_maps = _host_prep(voxel_features, batch_idx, y_idx, x_idx)
    r = _get_runner()
    r.set_inputs(in_maps)
    r.run()
    r.block()
    slabs = r.fetch("out")  # [8, 128, 80000] bf16
    out = np.empty((B, C, NY, NX), np.float32)
    for k in range(N_CORES):
        b, h = k // 2, k % 2
        out[b, :, h * 200:(h + 1) * 200, :] = \
            slabs[k].astype(np.float32).reshape(C, 200, NX)
    return out


def time_kernel(n_iters=33, reps=5):
    """Slope-time the device body: returns est. HW ns per body iteration."""
    import time as _time
    r = _get_runner()
    assert r._dev_inputs is not None, "call kernel() first"

    def run_with_nit(n):
        r.update_input("nit", [np.array([[n, 0]], np.int32)] * N_CORES)
        r.run(); r.block()
        ts = []
        for _ in range(reps):
            t0 = _time.perf_counter()
            r.run(); r.block()
            ts.append(_time.perf_counter() - t0)
        return min(ts)

    t1 = run_with_nit(1)
    tn = run_with_nit(n_iters)
    r.update_input("nit", [np.array([[1, 0]], np.int32)] * N_CORES)
    return (tn - t1) / (n_iters - 1) * 1e9, t1, tn
